# revision 1
# baseline (speedup 1.0000x reference)
"""ByteEncoder Trainium2 kernel.

Model: h = embed[x]; y = Conv1d(k=4, s=4)(h); y = LN(y)*g+b; xb = y@bW.T+bb;
       h_t = lam*h_{t-1} + (1-lam)*xb_t (LRU scan); out = h@cW.T+cb.

Strategy (8 NeuronCores, data-parallel over (batch, half-sequence)):
  * embed+conv fused into 4 lookup tables LUT_j[v,o] = sum_d embed[v,d]*conv_w[o,d,j],
    computed on-device by GEMM; conv becomes y^T = sum_j LUT_j^T @ onehot_j with
    onehot built from x on-chip. Everything is kept channel-major [d, t] so the
    LRU scan maps onto the DVE tensor_tensor_scan instruction (state = lam*state + u
    along the free axis), and LN stats are computed with ones-vector matmuls.
  * Sequence halves are chained: every core scans with initial state 0, final
    states are AllGather'd within each (batch) pair, and the second-half core
    adds lam^{t+1} * carry (lam powers via ACT exp((t+1)*ln(lam))).
  * Host side only reshapes/transposes/slices inputs (layout marshalling); all
    FLOPs (LUT GEMM, conv, LN, projections, scan, lam transcendentals) are on-device.
  * Matmuls run as float32r (full fp32 data, 1 cycle/row PE mode at N>=256).
"""

import sys

sys.path.insert(0, "/opt/trn_rl_repo")

from contextlib import ExitStack

import numpy as np

import concourse.bass as bass
import concourse.tile as tile
from concourse import mybir

B, T, D = 4, 8192, 1024
NCORES = 8
TC = T // 4            # 2048 conv tokens per batch
TPC = TC // 2          # 1024 conv tokens per core
XPC = TPC * 4          # 4096 input tokens per core
V = 256                # vocab
P = 128
DT = D // P            # 8 d-tiles (also o-tiles)
VT = V // P            # 2 v-tiles
NJ = 4                 # conv taps
NT = TPC // 512        # 2 t-chunks of 512
NO = D // 512          # 2 o-chunks of 512

F32 = mybir.dt.float32
F32R = mybir.dt.float32r
I32 = mybir.dt.int32
AF = mybir.ActivationFunctionType
OP = mybir.AluOpType

LN_EPS = 1e-5


def _vec_view(dram_ap):
    """[D] dram vector -> [128, 8] view (partition p, free dt; d = dt*128+p)."""
    return dram_ap.rearrange("(dt p) -> p dt", p=P)


def build_nc():
    nc = bass.Bass(trn_type="TRN2", num_devices=NCORES)

    x_i = nc.declare_dram_parameter("x_i", [XPC], I32, isOutput=False)
    embedT = nc.declare_dram_parameter("embedT", [D, V], F32, isOutput=False)
    convwT = nc.declare_dram_parameter("convwT", [NJ, D, D], F32, isOutput=False)
    bWT = nc.declare_dram_parameter("bWT", [D, D], F32, isOutput=False)
    cWT = nc.declare_dram_parameter("cWT", [D, D], F32, isOutput=False)
    conv_b = nc.declare_dram_parameter("conv_b", [D], F32, isOutput=False)
    ln_g = nc.declare_dram_parameter("ln_g", [D], F32, isOutput=False)
    ln_b = nc.declare_dram_parameter("ln_b", [D], F32, isOutput=False)
    log_lambda = nc.declare_dram_parameter("log_lambda", [D], F32, isOutput=False)
    bb = nc.declare_dram_parameter("bb", [D], F32, isOutput=False)
    cb = nc.declare_dram_parameter("cb", [D], F32, isOutput=False)
    parity = nc.declare_dram_parameter("parity", [1], F32, isOutput=False)
    out = nc.declare_dram_parameter("out", [TPC, D], F32, isOutput=True)

    fin_dram = nc.dram_tensor("fin_dram", [D], F32)
    fin_all = nc.dram_tensor("fin_all", [2, D], F32)

    with tile.TileContext(nc) as tc, ExitStack() as ctx, \
            nc.allow_low_precision(reason="float32r matmul operands"):
        _body(ctx, tc, x_i.ap(), embedT.ap(), convwT.ap(), bWT.ap(), cWT.ap(),
              conv_b.ap(), ln_g.ap(), ln_b.ap(), log_lambda.ap(), bb.ap(),
              cb.ap(), parity.ap(), out.ap(), fin_dram.ap(), fin_all.ap())
    _split_excess_waits(nc)
    return nc


def _split_excess_waits(nc, max_waits=1):
    """walrus codegen allows only one sync-wait slot per TPB instruction;
    hoist excess waits onto single-wait NoOps inserted just before the
    instruction on the same engine queue (queue order makes this exact)."""
    cnt = 0
    for f in nc.m.functions:
        for b in f.blocks:
            insts = list(b.instructions)
            out_list = []
            for inst in insts:
                si = inst.sync_info
                waits = list(si.on_wait) if si is not None and si.on_wait else []
                if len(waits) > max_waits:
                    for w in waits[:-max_waits]:
                        nop = mybir.InstNoOp(
                            name=f"waitsplit_{cnt}",
                            sync_info=mybir.SyncInfo(on_wait=[w], on_update=[]),
                        )
                        nop.engine = inst.engine
                        nc.inst_map[nop.name] = nop
                        cnt += 1
                        out_list.append(nop)
                    inst.sync_info = mybir.SyncInfo(
                        on_wait=waits[-max_waits:],
                        on_update=list(si.on_update) if si.on_update else [])
                out_list.append(inst)
            b.instructions = out_list
    return cnt


def _body(ctx, tc, x_i, embedT, convwT, bWT, cWT, conv_b, ln_g, ln_b,
          log_lambda, bb, cb, parity, out, fin_dram, fin_all):
    nc = tc.nc

    # Five banks of 8 slots of [128, 1024] f32 (4KB/partition each), reused
    # across phases so peak SBUF stays under budget:
    #   S1: onehot (ph0-B)   -> ys (C-D)     -> h (E-F)
    #   S2: x_bcast (ph0)    -> bWT (B-D)    -> lam_pow (E)
    #   S3: LUT (A-B)        -> cWT (C-F)
    #   S4: y (B-C)          -> u (D-E)
    big = ctx.enter_context(tc.tile_pool(name="big", bufs=1))
    small = ctx.enter_context(tc.tile_pool(name="small", bufs=1))
    stream = ctx.enter_context(tc.tile_pool(name="stream", bufs=3))
    y2pool = ctx.enter_context(tc.tile_pool(name="y2pool", bufs=2))
    ppool = ctx.enter_context(tc.tile_pool(name="ppool", bufs=3, space="PSUM"))
    pbig = ctx.enter_context(tc.tile_pool(name="pbig", bufs=2, space="PSUM"))
    pstat = ctx.enter_context(tc.tile_pool(name="pstat", bufs=1, space="PSUM"))

    _uid = [0]

    def big_tile(slot, shape=(P, 1024), dtype=F32):
        _uid[0] += 1
        tag = f"slot_{slot[0]}_{slot[1]}"
        return big.tile(list(shape), dtype, tag=tag, name=f"{tag}_{_uid[0]}")

    # ---------------- phase 0: constants, x broadcast, onehot ----------------
    convb_t = small.tile([P, DT], F32, tag="convb")
    g_t = small.tile([P, DT], F32, tag="g")
    lnb_t = small.tile([P, DT], F32, tag="lnb")
    ll_t = small.tile([P, DT], F32, tag="ll")
    bb_t = small.tile([P, DT], F32, tag="bb")
    nc.sync.dma_start(out=convb_t, in_=_vec_view(conv_b))
    nc.sync.dma_start(out=g_t, in_=_vec_view(ln_g))
    nc.sync.dma_start(out=lnb_t, in_=_vec_view(ln_b))
    nc.sync.dma_start(out=ll_t, in_=_vec_view(log_lambda))
    nc.sync.dma_start(out=bb_t, in_=_vec_view(bb))
    cb_bc = small.tile([P, D], F32, tag="cb")
    nc.sync.dma_start(out=cb_bc, in_=cb.partition_broadcast(P))
    parity_sb = small.tile([P, 1], F32, tag="parity")
    nc.sync.dma_start(out=parity_sb, in_=parity.partition_broadcast(P))

    # lam = sigmoid(exp(log_lambda)); also ln(lam), 1-lam
    e_t = small.tile([P, DT], F32, tag="e")
    lam_t = small.tile([P, DT], F32, tag="lam")
    ll2_t = small.tile([P, DT], F32, tag="ll2")
    oml_t = small.tile([P, DT], F32, tag="oml")
    nc.scalar.activation(out=e_t, in_=ll_t, func=AF.Exp)
    nc.scalar.activation(out=lam_t, in_=e_t, func=AF.Sigmoid)
    nc.scalar.activation(out=ll2_t, in_=lam_t, func=AF.Ln)
    nc.vector.tensor_scalar(out=oml_t, in0=lam_t, scalar1=-1.0, scalar2=1.0,
                            op0=OP.mult, op1=OP.add)

    ones_m32 = small.tile([P, P], F32, tag="ones_m32")
    nc.vector.memset(ones_m32, 1.0)
    ones_mat = small.tile([P, P], F32R, tag="ones_mat")  # all-ones colsum lhsT
    nc.vector.tensor_copy(out=ones_mat, in_=ones_m32)

    iota_v = small.tile([P, 1], I32, tag="iota_v")
    nc.gpsimd.iota(iota_v, [[0, 1]], base=0, channel_multiplier=1)
    iota_vf = small.tile([P, 1], F32, tag="iota_vf")
    nc.vector.tensor_copy(out=iota_vf, in_=iota_v)
    iota_vf2 = small.tile([P, 1], F32, tag="iota_vf2")
    nc.vector.tensor_scalar(out=iota_vf2, in0=iota_vf, scalar1=float(P),
                            scalar2=None, op0=OP.add)

    tpos_i = y2pool.tile([P, TPC], I32, tag="y2")
    nc.gpsimd.iota(tpos_i, [[1, TPC]], base=1, channel_multiplier=0)
    tpos_f = small.tile([P, TPC], F32, tag="tpos_f")
    nc.vector.tensor_copy(out=tpos_f, in_=tpos_i)

    # x broadcast across partitions, int32 -> f32, then onehot via is_equal
    Q = XPC // 4  # 1024 x-positions (256 conv tokens) per quarter tile
    xbc_i, xbc_f = [], []
    for q in range(4):
        t_ = big_tile(("S2", q), (P, Q), I32)
        nc.sync.dma_start(out=t_,
                          in_=x_i[q * Q:(q + 1) * Q].partition_broadcast(P))
        xbc_i.append(t_)
    for q in range(4):
        t_ = big_tile(("S2", 4 + q), (P, Q), F32)
        nc.vector.tensor_copy(out=t_, in_=xbc_i[q])
        xbc_f.append(t_)

    oh = {}
    for j in range(NJ):
        for vt in range(VT):
            o_t = big_tile(("S1", j * VT + vt), dtype=F32R)
            oh[(j, vt)] = o_t
            iv = iota_vf if vt == 0 else iota_vf2
            for q in range(4):
                # quarter q covers conv tokens [q*256, (q+1)*256); x index 4t+j
                xv = xbc_f[q].rearrange("p (t j) -> p t j", j=NJ)[:, :, j]
                nc.vector.tensor_scalar(out=o_t[:, q * 256:(q + 1) * 256],
                                        in0=xv, scalar1=iv, scalar2=None,
                                        op0=OP.is_equal)

    # ---------------- phase A: LUT_j[v, o] = embedT.T @ convwT_j ----------------
    et = big_tile(("S4", 0), (P, DT, V), dtype=F32R)  # 8KB; S4 slot 0 reused by y[0]
    nc.sync.dma_start(
        out=et, in_=embedT.bitcast(F32R).rearrange("(dt p) v -> p dt v", p=P))

    lut = {}
    for j in range(NJ):
        ps = [pbig.tile([P, 1024], F32, tag="pA", name=f"psA_{j}_{v_}")
              for v_ in range(VT)]
        for dt_ in range(DT):
            cw = stream.tile([P, D], F32R, tag="cw")
            nc.sync.dma_start(
                out=cw,
                in_=convwT[j].bitcast(F32R).rearrange("(dt p) o -> dt p o", p=P)[dt_])
            for vt in range(VT):
                for oc in range(NO):
                    nc.tensor.matmul(
                        ps[vt][:, oc * 512:(oc + 1) * 512],
                        et[:, dt_, vt * P:(vt + 1) * P],
                        cw[:, oc * 512:(oc + 1) * 512],
                        start=(dt_ == 0), stop=(dt_ == DT - 1))
        for vt in range(VT):
            l_t = big_tile(("S3", j * VT + vt), dtype=F32R)
            lut[(j, vt)] = l_t
            nc.scalar.activation(out=l_t, in_=ps[vt], func=AF.Copy)

    # bWT tiles arrive during A/B (S2 slots free once onehot is built)
    bwt = []
    for dt_ in range(DT):
        t_ = big_tile(("S2", dt_), dtype=F32R)
        bwt.append(t_)
        nc.sync.dma_start(
            out=t_, in_=bWT.bitcast(F32R).rearrange("(dt p) o -> dt p o", p=P)[dt_])

    # b2[o] = sum_d bW[o, d] * ln_b[d]  (uses unfolded bWT), out [128, 8]
    b2_t = small.tile([P, DT], F32, tag="b2")
    for ot in range(DT):
        psb = ppool.tile([P, 1], F32, tag="pB", name=f"psb{ot}")
        for dt_ in range(DT):
            nc.tensor.matmul(psb, bwt[dt_][:, ot * P:(ot + 1) * P].bitcast(F32),
                             lnb_t[:, dt_:dt_ + 1],
                             start=(dt_ == 0), stop=(dt_ == DT - 1))
        nc.vector.tensor_copy(out=b2_t[:, ot:ot + 1], in_=psb)
    # c0 = (1-lam) * (b2 + bb)
    c0_t = small.tile([P, DT], F32, tag="c0")
    nc.vector.tensor_add(out=c0_t, in0=b2_t, in1=bb_t)
    nc.vector.tensor_mul(out=c0_t, in0=c0_t, in1=oml_t)

    # fold ln_g into bWT rows: W_g^T[d, o] = bWT[d, o] * g[d]
    for dt_ in range(DT):
        nc.vector.tensor_scalar(out=bwt[dt_], in0=bwt[dt_][:, :].bitcast(F32),
                                scalar1=g_t[:, dt_:dt_ + 1], scalar2=None,
                                op0=OP.mult)

    # ---------------- phase B: conv GEMM  y^T[o, t] ----------------
    y = []
    for ot in range(DT):
        y_t = big_tile(("S4", ot), dtype=F32R)
        y.append(y_t)
        for tc_ in range(NT):
            psy = ppool.tile([P, 512], F32, tag="pB")
            kk = 0
            for j in range(NJ):
                for vt in range(VT):
                    nc.tensor.matmul(
                        psy,
                        lut[(j, vt)][:, ot * P:(ot + 1) * P],
                        oh[(j, vt)][:, tc_ * 512:(tc_ + 1) * 512],
                        start=(kk == 0), stop=(kk == NJ * VT - 1))
                    kk += 1
            # y = psum + conv_b (per-partition bias on ACT)
            nc.scalar.activation(out=y_t[:, tc_ * 512:(tc_ + 1) * 512], in_=psy,
                                 func=AF.Identity, bias=convb_t[:, ot:ot + 1],
                                 scale=1.0)

    # cWT tiles arrive during B/C (S3 slots free after conv GEMM)
    cwt = []
    for dt_ in range(DT):
        t_ = big_tile(("S3", dt_), dtype=F32R)
        cwt.append(t_)
        nc.sync.dma_start(
            out=t_, in_=cWT.bitcast(F32R).rearrange("(dt p) o -> dt p o", p=P)[dt_])

    # ---------------- phase C: LayerNorm stats + normalize ----------------
    # ones_mat lhsT makes the column sums come out replicated across all 128
    # partitions, so mu/rstd are directly usable by elementwise engines.
    # Two sequential passes (sum, then square+sum) keep PSUM at 2 banks.
    eps_sb = small.tile([P, 1], F32, tag="eps")
    nc.vector.memset(eps_sb, LN_EPS)
    mub_sb = small.tile([P, TPC], F32, tag="mub")     # mean, replicated
    rb_sb = small.tile([P, TPC], F32, tag="rb")       # rstd, replicated
    for tc_ in range(NT):
        sl = slice(tc_ * 512, (tc_ + 1) * 512)
        ps_s = pstat.tile([P, 512], F32, tag="pS", name=f"ps_s{tc_}")
        for ot in range(DT):
            nc.tensor.matmul(ps_s, ones_mat, y[ot][:, sl],
                             start=(ot == 0), stop=(ot == DT - 1))
        nc.scalar.activation(out=mub_sb[:, sl], in_=ps_s, func=AF.Copy,
                             scale=1.0 / D)
    for tc_ in range(NT):
        sl = slice(tc_ * 512, (tc_ + 1) * 512)
        ps_q = pstat.tile([P, 512], F32, tag="pS", name=f"ps_q{tc_}")
        for ot in range(DT):
            y2 = y2pool.tile([P, 512], F32R, tag="y2", name=f"y2_{tc_}_{ot}")
            nc.scalar.activation(out=y2, in_=y[ot][:, sl].bitcast(F32),
                                 func=AF.Square)
            nc.tensor.matmul(ps_q, ones_mat, y2,
                             start=(ot == 0), stop=(ot == DT - 1))
        # var = E[y^2] - mu^2; rstd = 1/sqrt(var + eps)   (all replicated)
        nc.scalar.activation(out=rb_sb[:, sl], in_=ps_q, func=AF.Copy,
                             scale=1.0 / D)
        mu2 = y2pool.tile([P, 512], F32, tag="mu2", name=f"mu2_{tc_}")
        nc.vector.tensor_mul(out=mu2, in0=mub_sb[:, sl], in1=mub_sb[:, sl])
        nc.vector.tensor_sub(out=rb_sb[:, sl], in0=rb_sb[:, sl], in1=mu2)
        nc.scalar.activation(out=rb_sb[:, sl], in_=rb_sb[:, sl], func=AF.Sqrt,
                             bias=eps_sb)
        nc.vector.reciprocal(out=rb_sb[:, sl], in_=rb_sb[:, sl])

    # ys = (y - mu) * rstd   (sub on gpsimd, mul on DVE)
    ys = []
    for ot in range(DT):
        tmp = y2pool.tile([P, TPC], F32, tag="ntmp")
        nc.gpsimd.tensor_sub(out=tmp, in0=y[ot][:, :].bitcast(F32), in1=mub_sb)
        ys_t = big_tile(("S1", ot), dtype=F32R)
        ys.append(ys_t)
        nc.vector.tensor_mul(out=ys_t, in0=tmp, in1=rb_sb)

    # ---------------- phase D: b-projection + scan input ----------------
    u = []
    for ot in range(DT):
        u_t = big_tile(("S4", ot))
        u.append(u_t)
        for tc_ in range(NT):
            psx = ppool.tile([P, 512], F32, tag="pB")
            for dt_ in range(DT):
                nc.tensor.matmul(
                    psx, bwt[dt_][:, ot * P:(ot + 1) * P],
                    ys[dt_][:, tc_ * 512:(tc_ + 1) * 512],
                    start=(dt_ == 0), stop=(dt_ == DT - 1))
            # u = (1-lam) * xb + c0
            nc.scalar.activation(out=u_t[:, tc_ * 512:(tc_ + 1) * 512], in_=psx,
                                 func=AF.Identity, scale=oml_t[:, ot:ot + 1],
                                 bias=c0_t[:, ot:ot + 1])

    # ---------------- phase E: LRU scan + cross-core chaining ----------------
    h_sb = []
    finals = small.tile([P, DT], F32, tag="finals")
    for dt_ in range(DT):
        h_t = big_tile(("S1", dt_), dtype=F32R)
        h_sb.append(h_t)
        lam_bc = lam_t[:, dt_:dt_ + 1].broadcast_to((P, TPC))
        nc.vector.tensor_tensor_scan(out=h_t, data0=lam_bc, data1=u[dt_],
                                     initial=0.0, op0=OP.mult, op1=OP.add)
        nc.gpsimd.tensor_copy(out=finals[:, dt_:dt_ + 1],
                              in_=h_t[:, TPC - 1:TPC].bitcast(F32))

    # exchange final states within each (batch) pair
    nc.sync.dma_start(out=fin_dram.rearrange("(dt p) -> p dt", p=P), in_=finals)
    nc.gpsimd.collective_compute(
        "AllGather", OP.bypass,
        replica_groups=[[0, 1], [2, 3], [4, 5], [6, 7]],
        ins=[fin_dram], outs=[fin_all])
    carry = small.tile([P, DT], F32, tag="carry")
    nc.sync.dma_start(out=carry, in_=fin_all[0].rearrange("(dt p) -> p dt", p=P))
    # mask: only odd (second-half) cores apply the carry
    nc.vector.tensor_scalar(out=carry, in0=carry, scalar1=parity_sb,
                            scalar2=None, op0=OP.mult)

    # lam^{t+1} table and correction h += lam^{t+1} * carry
    for dt_ in range(DT):
        lp = big_tile(("S2", dt_))
        nc.scalar.activation(out=lp, in_=tpos_f, func=AF.Exp,
                             scale=ll2_t[:, dt_:dt_ + 1])
        nc.vector.scalar_tensor_tensor(
            out=h_sb[dt_], in0=lp, scalar=carry[:, dt_:dt_ + 1],
            in1=h_sb[dt_][:, :].bitcast(F32), op0=OP.mult, op1=OP.add)

    # ---------------- phase F: c-projection + cb, DMA out ----------------
    for tt in range(DT):
        for oc in range(NO):
            pso = ppool.tile([P, 512], F32, tag="pB")
            for dt_ in range(DT):
                nc.tensor.matmul(
                    pso, h_sb[dt_][:, tt * P:(tt + 1) * P],
                    cwt[dt_][:, oc * 512:(oc + 1) * 512],
                    start=(dt_ == 0), stop=(dt_ == DT - 1))
            # epilogue adds cb while moving PSUM->SBUF staging
            ostage = stream.tile([P, 512], F32, tag="ostage", name=f"ost{tt}_{oc}")
            nc.vector.scalar_tensor_tensor(
                out=ostage, in0=pso, scalar=1.0,
                in1=cb_bc[:, oc * 512:(oc + 1) * 512],
                op0=OP.mult, op1=OP.add)
            nc.sync.dma_start(
                out=out[tt * P:(tt + 1) * P, oc * 512:(oc + 1) * 512], in_=ostage)


_NC_CACHE = None


def _get_nc():
    global _NC_CACHE
    if _NC_CACHE is None:
        _NC_CACHE = build_nc()
    return _NC_CACHE


def _in_maps(x, embed, conv_w, conv_b, ln_g, ln_b, log_lambda, bW, bb, cW, cb):
    f = lambda a: np.ascontiguousarray(np.asarray(a, dtype=np.float32))
    x = np.asarray(x)
    embedT = f(np.asarray(embed, np.float32).T)                  # [D, V]
    convwT = f(np.asarray(conv_w, np.float32).transpose(2, 1, 0))  # [j, d, o]
    bWT = f(np.asarray(bW, np.float32).T)                        # [d, o]
    cWT = f(np.asarray(cW, np.float32).T)                        # [d, o]
    shared = dict(embedT=embedT, convwT=convwT, bWT=bWT, cWT=cWT,
                  conv_b=f(conv_b), ln_g=f(ln_g), ln_b=f(ln_b),
                  log_lambda=f(log_lambda), bb=f(bb), cb=f(cb))
    maps = []
    for c in range(NCORES):
        b, h = c // 2, c % 2
        xi = np.ascontiguousarray(
            x[b, h * XPC:(h + 1) * XPC].astype(np.int32))
        maps.append(dict(x_i=xi, parity=np.array([float(h)], np.float32),
                         **shared))
    return maps


def _unshard(results):
    out = np.empty((B, TC, D), np.float32)
    for c in range(NCORES):
        b, h = c // 2, c % 2
        out[b, h * TPC:(h + 1) * TPC, :] = results[c]["out"]
    return out


def run(trace=False, **inputs):
    from concourse.bass_utils import run_bass_kernel_spmd
    nc = _get_nc()
    maps = _in_maps(**inputs)
    res = run_bass_kernel_spmd(nc, maps, list(range(NCORES)), trace=trace)
    return _unshard(res.results), res


def kernel(**inputs):
    out, _ = run(trace=False, **inputs)
    return out

